# revision 1
# baseline (speedup 1.0000x reference)
"""Self-contained Trainium2 Bass kernel for nn_Encoder (causal MHA).

The reference computes softmax(QK^T/sqrt(D)) * tril, renormalized — which is
exactly a causal softmax (the full-row max/normalizer cancel) — followed by
P@V, head concat and an output projection. The time-decay branch is dead code,
so the whole module reduces to causal multi-head attention.

Sharding: 16 (batch, head) pairs across 8 cores, 2 heads of one batch per
core. Each core computes partial_out = sum_h attn_h(X) @ wO_h for its two
heads ([2048, 256]); the host adds the 4 core-partials per batch element.

Host-side algebra folds (exact, fp64 on host):
  scores = Q K^T = X (wQ wK^T) X^T   -> ship A^T = wK wQ^T; ONE score-side
                                        projection KAT = A^T_proj @ X^T and
                                        X^T itself is the moving operand
  (P @ V) @ wO_h = P @ (X (wV wO_h)) -> ship wV wO_h fused; the PV product
                                        directly yields output rows

Per-core dataflow (everything fp32-in-SBUF; matmul operands are float32r =
fp32 with 12-bit mantissa at 4x PE throughput, rounded by the producing
DVE/GpSimd op as the ISA requires):
  scores are computed transposed, S^T[j, i] = KAT.T @ X^T, so the softmax
  needs no transposes: exp runs on ACT straight out of PSUM, the causal mask
  (+f32r rounding) is one DVE mul per diagonal chunk, and P^T chunks are
  exactly the PV matmul's stationary operand. V tiles carry an appended ones
  column so each PV accumulation also emits the softmax row-sums for free;
  1/rowsum is applied during the PSUM->SBUF copy (head 0) or fused with the
  cross-head add via scalar_tensor_tensor (head 1). Diagonal chunks are
  trimmed to their causal half where the f32r moving-dim >= 256 constraint
  allows, and fully-masked PV sub-blocks are skipped. A 4-chunk software
  pipeline (ST -> exp -> round -> PV) keeps all engines overlapped.
"""

from contextlib import ExitStack

import numpy as np

B, S, D, H = 2, 2048, 256, 8
N_CORES = 8
P = 128          # partition size
SG = 512         # query group (i) width
NSG = S // SG    # 4 query groups
NJC = S // P     # 16 key chunks
EC = D // P      # 2 chunks along the head dim e
DC = D // P      # 2 chunks along the model dim d
VW = 258         # V tile width: 256 e cols + ones col + zero pad (even for f32r)

# Matmul operand dtype knob: "f32" (exact, 4 cyc/row) or "f32r" (fp32 with
# 12-bit mantissa, 1 cyc/row — 2.7x faster end-to-end, ~2e-4 rel err).
import os as _os
MM_DT = _os.environ.get("MM_DT", "f32r")

_STATE = {}


def _build_nc():
    import concourse.tile as tile
    from concourse import bacc, mybir

    f32 = mybir.dt.float32
    f32r = mybir.dt.float32r
    attn_dt = f32r if MM_DT == "f32r" else f32
    proj_dt = attn_dt

    def cast(ap, kind):
        return ap

    nc = bacc.Bacc("TRN2", target_bir_lowering=False, debug=False,
                   num_devices=N_CORES)

    xt_d = nc.dram_tensor("xt", [D, S], f32, kind="ExternalInput")
    wa_d = nc.dram_tensor("wa", [2, P, DC, D], f32, kind="ExternalInput")
    wvo_d = nc.dram_tensor("wvo", [2, P, DC, D], f32, kind="ExternalInput")
    mask_d = nc.dram_tensor("mask", [P, 4 * SG], f32, kind="ExternalInput")
    out_d = nc.dram_tensor("out", [S, D], f32, kind="ExternalOutput")

    with tile.TileContext(nc) as tc, ExitStack() as ctx:
        pool = lambda name, bufs, **kw: ctx.enter_context(
            tc.tile_pool(name=name, bufs=bufs, **kw))
        consts = pool("consts", 1)
        xtp = pool("xt", 2)
        stg = pool("stg", 2)
        wts = pool("wts", 4)
        qkp = pool("qk", 4)
        vp = pool("v", 2 * NJC)
        ptp = pool("pt", 5)
        rsp = pool("rs", 8)
        retp = pool("ret", NJC + 8)
        outp = pool("outsb", 4)
        ps_big = pool("ps_big", 4, space="PSUM")
        ps_acc = pool("ps_acc", 4, space="PSUM")

        # Weights first (small; the first projection needs them), then xt in
        # [P, SG] strips, sg-major, so the first KA-projection matmul can
        # start after ~3 strips instead of waiting out the full 2MB load.
        w_sb = {}

        def load_w(name, dram, h):
            t = wts.tile([P, DC, D], proj_dt, tag="w", name="wt")
            if proj_dt == f32:
                nc.sync.dma_start(out=t[:], in_=dram[h])
            else:
                raw = stg.tile([P, DC, D], f32, tag="wstg", name="wraw")
                nc.sync.dma_start(out=raw[:], in_=dram[h])
                nc.vector.tensor_copy(out=t[:], in_=raw[:])
            w_sb[name, h] = t

        # Interleave the loads so the first projection's operands (wa h0 +
        # the sg0 xt strips) arrive first; xt strips go out on the otherwise
        # idle GpSimd DMA queue, weights/mask on Sync, so issue overlaps.
        load_w("wa", wa_d, 0)
        xt_sb = [xtp.tile([P, S], proj_dt, tag="xt", name="xt_sb")
                 for _ in range(DC)]
        for sg in range(NSG):
            for dc in range(DC):
                w0, w1 = sg * SG, (sg + 1) * SG
                if proj_dt == f32:
                    nc.gpsimd.dma_start(out=xt_sb[dc][:, w0:w1],
                                        in_=xt_d[dc * P:(dc + 1) * P, w0:w1])
                else:
                    raw = stg.tile([P, SG], f32, tag="stg", name="xtraw")
                    nc.gpsimd.dma_start(out=raw[:],
                                        in_=xt_d[dc * P:(dc + 1) * P, w0:w1])
                    nc.vector.tensor_copy(out=xt_sb[dc][:, w0:w1], in_=raw[:])
            if sg == 0:
                load_w("wvo", wvo_d, 0)
            elif sg == 1:
                mask_sb = consts.tile([P, 4 * SG], f32)
                nc.sync.dma_start(out=mask_sb[:], in_=mask_d[:])
            elif sg == 2:
                load_w("wa", wa_d, 1)
            else:
                load_w("wvo", wvo_d, 1)

        ret0 = {}  # ic -> head0 normalized output chunk [P, D]

        for h in range(2):
            # --- projections: KAT = (wK wQ^T) @ X^T and V' = X (wV wO_h) ---
            # scores come from X A X^T (A = wQ wK^T folded on the host), so
            # only ONE score-side projection is needed; X^T itself is the
            # moving operand of the score matmuls.
            kt = [qkp.tile([P, S], attn_dt, tag="qk", name="kt")
                  for _ in range(EC)]
            w = w_sb["wa", h]
            for sg in range(NSG):
                for ec in range(EC):
                    ps = ps_big.tile([P, SG], f32, tag="big")
                    for dc in range(DC):
                        nc.tensor.matmul(
                            ps[:],
                            cast(w[:, dc, ec * P:(ec + 1) * P], "proj"),
                            cast(xt_sb[dc][:, sg * SG:(sg + 1) * SG],
                                 "proj"),
                            start=(dc == 0), stop=(dc == DC - 1))
                    nc.vector.tensor_copy(
                        out=kt[ec][:, sg * SG:(sg + 1) * SG], in_=ps[:])
            v_sb = []
            wv = w_sb["wvo", h]
            for jc in range(NJC):
                ps = ps_acc.tile([P, VW], f32, tag="acc")
                for dc in range(DC):
                    nc.tensor.matmul(
                        ps[:, 0:D],
                        cast(xt_sb[dc][:, jc * P:(jc + 1) * P], "proj"),
                        cast(wv[:, dc, :], "proj"),
                        start=(dc == 0), stop=(dc == DC - 1))
                vt = vp.tile([P, VW], attn_dt, tag="v")
                nc.gpsimd.memset(vt[:, D:D + 1].bitcast(f32), 1.0)
                nc.gpsimd.memset(vt[:, D + 1:VW].bitcast(f32), 0.0)
                nc.vector.tensor_copy(out=vt[:, 0:D], in_=ps[:, 0:D])
                v_sb.append(vt)

            # --- attention: S^T chunks -> exp -> (mask/round) -> PV accum.
            # Diagonal chunks are trimmed: chunk t only needs i >= t*128, so
            # compute columns [c0, SG) with c0 = min(t,2)*128 (c0 capped so
            # the f32r moving dim stays >= 256) and skip PV sub-blocks ib < t.
            for qo in range(NSG):
                njc = (qo + 1) * 4
                po = [ps_acc.tile([P, VW], f32, tag="acc", name="po") for _ in range(4)]

                def emit_pv(pjc, ppt, t):
                    for ib in range(max(t, 0), 4):
                        nc.tensor.matmul(
                            po[ib][:],
                            cast(ppt[:, ib * P:(ib + 1) * P], "attn"),
                            cast(v_sb[pjc][:], "attn"),
                            start=(pjc == 0), stop=(ib == t))

                pending = []
                for jc in range(njc):
                    t = jc - qo * 4
                    c0 = min(t, 2) * P if t > 0 else 0
                    ps = ps_big.tile([P, SG], f32, tag="big")
                    for ec in range(EC):
                        nc.tensor.matmul(
                            ps[:, c0:SG],
                            cast(kt[ec][:, jc * P:(jc + 1) * P], "attn"),
                            cast(xt_sb[ec][:, qo * SG + c0:(qo + 1) * SG],
                                 "attn"),
                            start=(ec == 0), stop=(ec == EC - 1))
                    pt = ptp.tile([P, SG], f32, tag="pt")
                    nc.scalar.activation(
                        out=pt[:, c0:SG], in_=ps[:, c0:SG],
                        func=mybir.ActivationFunctionType.Exp, scale=1.0 / 16.0)
                    if attn_dt == f32:
                        if t >= 0:
                            nc.vector.tensor_mul(
                                pt[:, c0:SG], pt[:, c0:SG],
                                mask_sb[:, t * SG + c0:(t + 1) * SG])
                        pv_src = pt
                    else:
                        # ACT can't emit f32r; round on DVE (the diagonal
                        # chunks fuse the causal mask into the rounding mul).
                        ptr = ptp.tile([P, SG], attn_dt, tag="ptr", name="ptr")
                        if t >= 0:
                            nc.vector.tensor_mul(
                                ptr[:, c0:SG], pt[:, c0:SG],
                                mask_sb[:, t * SG + c0:(t + 1) * SG])
                        else:
                            nc.vector.tensor_copy(out=ptr[:], in_=pt[:])
                        pv_src = ptr
                    pending.append((jc, pv_src, t))
                    if len(pending) > 4:
                        pjc, ppt, pp_t = pending.pop(0)
                        emit_pv(pjc, ppt, pp_t)
                while pending:
                    pjc, ppt, pp_t = pending.pop(0)
                    emit_pv(pjc, ppt, pp_t)

                # normalize: out_h = po[:, :D] * (1 / po[:, D]); since wO
                # is folded into the V projection these ARE output rows.
                for ib in range(4):
                    ic = qo * 4 + ib
                    rs_t = rsp.tile([P, 1], f32, tag="rs")
                    nc.vector.reciprocal(out=rs_t[:], in_=po[ib][:, D:D + 1])
                    if h == 0:
                        ret_t = retp.tile([P, D], f32, tag="ret")
                        nc.vector.tensor_scalar_mul(ret_t[:], po[ib][:, 0:D],
                                                    rs_t[:])
                        ret0[ic] = ret_t
                    else:
                        ob = outp.tile([P, D], f32, tag="out")
                        nc.vector.scalar_tensor_tensor(
                            out=ob[:], in0=po[ib][:, 0:D], scalar=rs_t[:],
                            in1=ret0[ic][:], op0=mybir.AluOpType.mult,
                            op1=mybir.AluOpType.add)
                        nc.sync.dma_start(out=out_d[ic * P:(ic + 1) * P, :],
                                          in_=ob[:])

    nc.compile()
    return nc


def _make_mask():
    # mask[r, t*SG + c] = 1 if (t*P + r) <= c else 0  (keep j <= i)
    r = np.arange(P)[:, None]
    c = np.arange(SG)[None, :]
    blocks = [((t * P + r) <= c).astype(np.float32) for t in range(4)]
    return np.concatenate(blocks, axis=1)


def _in_maps(inputs, wQ, wK, wV, wO):
    mask = _make_mask()
    maps = []
    for core in range(N_CORES):
        b = core // 4
        h0 = 2 * (core % 4)
        hs = [h0, h0 + 1]
        xt = np.ascontiguousarray(inputs[b].T)  # [D, S]
        wa = np.stack([
            (wK[h].astype(np.float64) @ wQ[h].astype(np.float64).T)
            .astype(np.float32).reshape(DC, P, D).transpose(1, 0, 2)
            for h in hs])
        wvo = np.stack([
            (wV[h].astype(np.float64)
             @ wO[h * D:(h + 1) * D, :].astype(np.float64))
            .astype(np.float32).reshape(DC, P, D).transpose(1, 0, 2)
            for h in hs])
        maps.append({
            "xt": xt,
            "wa": np.ascontiguousarray(wa),
            "wvo": np.ascontiguousarray(wvo),
            "mask": mask,
        })
    return maps


def _run(inputs, wQ, wK, wV, wO, trace=False, tmpdir=None):
    import time

    from concourse.bass_utils import run_bass_kernel_spmd

    if "nc" not in _STATE:
        _STATE["nc"] = _build_nc()
    maps = _in_maps(inputs, wQ, wK, wV, wO)
    res = None
    for attempt in range(4):
        try:
            res = run_bass_kernel_spmd(_STATE["nc"], maps,
                                       list(range(N_CORES)),
                                       trace=trace, tmpdir=tmpdir)
            break
        except Exception:
            # Transient NRT device faults (NRT_EXEC_UNIT_UNRECOVERABLE) have
            # been observed on the first execution of a fresh executable.
            # The PJRT client stays wedged in-process, so reset the backend
            # before retrying.
            if attempt == 3:
                raise
            try:
                import jax.extend.backend

                jax.extend.backend.clear_backends()
            except Exception:
                pass
            time.sleep(3.0)
    out = np.zeros((B, S, D), dtype=np.float32)
    for core in range(N_CORES):
        out[core // 4] += res.results[core]["out"]
    return out, res


def kernel(inputs, timestamp, wQ, wK, wV, wO, theta):
    inputs = np.asarray(inputs, dtype=np.float32)
    out, _ = _run(inputs, np.asarray(wQ, np.float32),
                  np.asarray(wK, np.float32), np.asarray(wV, np.float32),
                  np.asarray(wO, np.float32))
    return out


def kernel_profiled(inputs, timestamp, wQ, wK, wV, wO, theta, tmpdir=None):
    inputs = np.asarray(inputs, dtype=np.float32)
    out, res = _run(inputs, np.asarray(wQ, np.float32),
                    np.asarray(wK, np.float32), np.asarray(wV, np.float32),
                    np.asarray(wO, np.float32), trace=True, tmpdir=tmpdir)
    return out, res



# revision 2
# speedup vs baseline: 1.0001x; 1.0001x over previous
"""Self-contained Trainium2 Bass kernel for nn_Encoder (causal MHA), v2.

Reference reduces to causal multi-head attention (full-row softmax -> tril
mask -> renormalize == causal softmax; the time-decay branch is dead code).

Sharding: 16 (batch, head) pairs across 8 cores, 2 heads of one batch per
core. Each core computes partial_out = sum_h attn_h(X) @ wO_h for its two
heads ([2048, 256]); the host adds the 4 core-partials per batch element.

v2 vs the f32r baseline:
  - scores via ONE fp8e4m3 DoubleRow matmul per (key-chunk, query-group):
    contract d=256 in a single pass (2x the f32r column rate). kt8/xt8 are
    fp8 copies laid out [128, 2, S] with the d-chunk on the middle axis.
    Diagonal chunks get exact causal trim (fp8 has no >=256 moving-dim rule).
  - everything else in bf16 (was f32r): projections take host-shipped bf16
    weights and X^T (no on-device weight casts, half the DMA bytes); ACT
    writes exp output as bf16 directly, which deletes the off-diagonal
    DVE rounding pass entirely; causal masks are bf16 so the diagonal
    mask-mul runs in the DVE 2x mode; PV runs bf16 (1 cyc/col, same rate
    as f32r, no >=256 moving rule).
  - numerics (simulated end-to-end vs the fp64 reference on the harness
    inputs): rel_err ~1.24e-2, dominated by the one-term fp8 score quant.

Matmul cost on TRN2 is 1 output column per PE cycle per 128-contraction
(PSUM write port bound); fp8-DR contracts 256 per column pass. Per head:
proj 16384 + scores 17408 + PV 35088 cycles ~= 29.6us -> ~59us/core.
"""

from contextlib import ExitStack

import numpy as np

B, S, D, H = 2, 2048, 256, 8
N_CORES = 8
P = 128          # partition size
SG = 512         # query group (i) width
NSG = S // SG    # 4 query groups
NJC = S // P     # 16 key chunks
DC = D // P      # 2 chunks along the model dim d
VW = 258         # V tile width: 256 e cols + ones col + zero pad (odd widths are slow)

_STATE = {}


def _build_nc():
    import concourse.tile as tile
    from concourse import bacc, mybir

    f32 = mybir.dt.float32
    bf16 = mybir.dt.bfloat16
    f8 = mybir.dt.float8e4
    DR = mybir.MatmulPerfMode.DoubleRow

    nc = bacc.Bacc("TRN2", target_bir_lowering=False, debug=False,
                   num_devices=N_CORES)

    xt_d = nc.dram_tensor("xt", [DC, P, S], bf16, kind="ExternalInput")
    xt8_d = nc.dram_tensor("xt8", [P, DC, S], f8, kind="ExternalInput")
    wa_d = nc.dram_tensor("wa", [2, P, DC, D], bf16, kind="ExternalInput")
    wvo_d = nc.dram_tensor("wvo", [2, P, DC, D], bf16, kind="ExternalInput")
    mask_d = nc.dram_tensor("mask", [P, 4 * SG], bf16, kind="ExternalInput")
    out_d = nc.dram_tensor("out", [S, D], f32, kind="ExternalOutput")

    with tile.TileContext(nc) as tc, ExitStack() as ctx:
        pool = lambda name, bufs, **kw: ctx.enter_context(
            tc.tile_pool(name=name, bufs=bufs, **kw))
        consts = pool("consts", 1)
        xtp = pool("xt", 2)
        wts = pool("wts", 4)
        ktp = pool("kt", 2)
        vp = pool("v", 2 * NJC)
        ptp = pool("pt", 9)
        rsp = pool("rs", 8)
        retp = pool("ret", NJC + 8)
        outp = pool("outsb", 4)
        ps_big = pool("ps_big", 4, space="PSUM")
        ps_acc = pool("ps_acc", 4, space="PSUM")

        w_sb = {}

        def load_w(name, dram, h):
            t = wts.tile([P, DC, D], bf16, tag="w", name="wt")
            nc.sync.dma_start(out=t[:], in_=dram[h])
            w_sb[name, h] = t

        # xt (bf16) and xt8 (fp8 in DoubleRow [P, 2, S] layout) both come
        # straight from the host; xt in [P, SG] strips, sg-major, so the
        # first KA-projection can start after the first strips land.
        # wa(h0) sliced per (ec, dc) so the very first KA-projection matmul
        # waits on a 32KB slice, not the whole tensor; xt strips dc-ordered
        # the same way.
        wa0 = wts.tile([P, DC, D], bf16, tag="w", name="wt0")
        w_sb["wa", 0] = wa0
        for ec in range(DC):
            for dc in range(DC):
                nc.sync.dma_start(out=wa0[:, dc, ec * P:(ec + 1) * P],
                                  in_=wa_d[0, :, dc, ec * P:(ec + 1) * P])
        xt_sb = [xtp.tile([P, S], bf16, tag="xt", name="xt_sb")
                 for _ in range(DC)]
        xt8 = xtp.tile([P, DC, S], f8, tag="xt8", name="xt8")
        for sg in range(NSG):
            w0, w1 = sg * SG, (sg + 1) * SG
            for dc in range(DC):
                nc.gpsimd.dma_start(out=xt_sb[dc][:, w0:w1],
                                    in_=xt_d[dc, :, w0:w1])
            nc.gpsimd.dma_start(out=xt8[:, :, w0:w1],
                                in_=xt8_d[:, :, w0:w1])
            if sg == 0:
                load_w("wvo", wvo_d, 0)
            elif sg == 1:
                mask_sb = consts.tile([P, 4 * SG], bf16)
                nc.sync.dma_start(out=mask_sb[:], in_=mask_d[:])
            elif sg == 2:
                load_w("wa", wa_d, 1)
            else:
                load_w("wvo", wvo_d, 1)

        ret0 = {}  # ic -> head0 normalized output chunk [P, D]

        # Pre-create all V tiles (both heads) and set their ones/zero columns
        # during the input-DMA shadow.
        v_all = {}
        for h in range(2):
            for jc in range(NJC):
                vt = vp.tile([P, VW], bf16, tag="v", name="vt")
                # DVE is idle until the first projection lands; GpSimd is
                # busy issuing the xt DMAs, so the const columns go here.
                nc.vector.memset(vt[:, D:D + 1], 1.0)
                nc.vector.memset(vt[:, D + 1:VW], 0.0)
                v_all[h, jc] = vt

        kt8_h = {h: ktp.tile([P, DC, S], f8, tag="kt8", name="kt8")
                 for h in range(2)}

        def emit_katproj(h, sg):
            # KAT chunk: kt8[:, :, sg cols] = (wQ wK^T) @ X^T (bf16 mm)
            w = w_sb["wa", h]
            for ec in range(DC):
                ps = ps_big.tile([P, SG], f32, tag="big", name="pska")
                for dc in range(DC):
                    nc.tensor.matmul(
                        ps[:],
                        w[:, dc, ec * P:(ec + 1) * P],
                        xt_sb[dc][:, sg * SG:(sg + 1) * SG],
                        start=(dc == 0), stop=(dc == DC - 1))
                nc.vector.tensor_copy(
                    out=kt8_h[h][:, ec, sg * SG:(sg + 1) * SG], in_=ps[:])

        def emit_vproj(h, jc):
            wv = w_sb["wvo", h]
            ps = ps_big.tile([P, SG], f32, tag="big", name="psv")
            for dc in range(DC):
                nc.tensor.matmul(
                    ps[:, 0:D],
                    xt_sb[dc][:, jc * P:(jc + 1) * P],
                    wv[:, dc, :],
                    start=(dc == 0), stop=(dc == DC - 1))
            nc.vector.tensor_copy(out=v_all[h, jc][:, 0:D], in_=ps[:, 0:D])

        emit_katproj(0, 0)
        for jc in range(4):
            emit_vproj(0, jc)

        for h in range(2):
            kt8 = kt8_h[h]
            v_sb = [v_all[h, jc] for jc in range(NJC)]

            # --- attention: one fp8-DR score matmul per chunk -> exp(bf16)
            # -> (bf16 mask on diag) -> bf16 PV accumulation. Diagonal chunk
            # t only needs query cols >= t*128 (exact trim). Projection work
            # for the NEXT query group is interleaved into this group's
            # stream so the PE stays fed through the exp/mask latency.
            for qo in range(NSG):
                njc = (qo + 1) * 4
                po = [ps_acc.tile([P, VW], f32, tag="acc", name="po")
                      for _ in range(4)]

                def norm_ib(ib):
                    # normalize: out_h = po[:, :D] * (1 / po[:, D]); wO is
                    # folded into the V projection so these ARE output rows.
                    ic = qo * 4 + ib
                    rs_t = rsp.tile([P, 1], f32, tag="rs")
                    nc.vector.reciprocal(out=rs_t[:], in_=po[ib][:, D:D + 1])
                    if h == 0:
                        ret_t = retp.tile([P, D], f32, tag="ret")
                        nc.vector.tensor_scalar_mul(ret_t[:], po[ib][:, 0:D],
                                                    rs_t[:])
                        ret0[ic] = ret_t
                    else:
                        ob = outp.tile([P, D], f32, tag="out")
                        nc.vector.scalar_tensor_tensor(
                            out=ob[:], in0=po[ib][:, 0:D], scalar=rs_t[:],
                            in1=ret0[ic][:], op0=mybir.AluOpType.mult,
                            op1=mybir.AluOpType.add)
                        nc.sync.dma_start(
                            out=out_d[ic * P:(ic + 1) * P, :], in_=ob[:])

                def emit_pv(pjc, ppt, t):
                    for ib in range(max(t, 0), 4):
                        nc.tensor.matmul(
                            po[ib][:],
                            ppt[:, ib * P:(ib + 1) * P],
                            v_sb[pjc][:],
                            start=(pjc == 0), stop=(ib == t))
                        if ib == t:
                            norm_ib(ib)

                filler = []
                if qo < NSG - 1:
                    filler.append(lambda sg=qo + 1: emit_katproj(h, sg))
                    for jc in range(4 * qo + 4, 4 * qo + 8):
                        filler.append(lambda jc=jc: emit_vproj(h, jc))
                elif h == 0:
                    # prefetch head 1's first projections into head 0's tail
                    filler.append(lambda: emit_katproj(1, 0))
                    for jc in range(4):
                        filler.append(lambda jc=jc: emit_vproj(1, jc))

                pending = []
                for jc in range(njc):
                    t = jc - qo * 4
                    c0 = t * P if t > 0 else 0
                    ps = ps_big.tile([P, SG], f32, tag="big", name="pssc")
                    nc.tensor.matmul(
                        ps[:, c0:SG],
                        kt8[:, :, jc * P:(jc + 1) * P],
                        xt8[:, :, qo * SG + c0:(qo + 1) * SG],
                        start=True, stop=True, perf_mode=DR)
                    pt = ptp.tile([P, SG], bf16, tag="pt")
                    nc.scalar.activation(
                        out=pt[:, c0:SG], in_=ps[:, c0:SG],
                        func=mybir.ActivationFunctionType.Exp, scale=1.0 / 16.0)
                    if t >= 0:
                        ptm = ptp.tile([P, SG], bf16, tag="ptm", name="ptm")
                        nc.vector.tensor_mul(
                            ptm[:, c0:SG], pt[:, c0:SG],
                            mask_sb[:, t * SG + c0:(t + 1) * SG])
                        pv_src = ptm
                    else:
                        pv_src = pt
                    pending.append((jc, pv_src, t))
                    if filler:
                        filler.pop(0)()
                    if len(pending) > 6:
                        pjc, ppt, pp_t = pending.pop(0)
                        emit_pv(pjc, ppt, pp_t)
                while filler:
                    filler.pop(0)()
                while pending:
                    pjc, ppt, pp_t = pending.pop(0)
                    emit_pv(pjc, ppt, pp_t)

    nc.compile()
    return nc


def _make_mask():
    # mask[r, t*SG + c] = 1 if (t*P + r) <= c else 0  (keep key j <= query i)
    r = np.arange(P)[:, None]
    c = np.arange(SG)[None, :]
    blocks = [((t * P + r) <= c).astype(np.float32) for t in range(4)]
    return np.concatenate(blocks, axis=1)


def _in_maps(inputs, wQ, wK, wV, wO):
    import ml_dtypes as ml
    bf = ml.bfloat16
    mask = _make_mask().astype(bf)
    maps = []
    for core in range(N_CORES):
        b = core // 4
        h0 = 2 * (core % 4)
        hs = [h0, h0 + 1]
        xt = np.ascontiguousarray(
            inputs[b].T.astype(bf).reshape(DC, P, S))
        # fp8 copy of the bf16 X^T, DoubleRow layout [P, dc, S]
        xt8 = np.ascontiguousarray(
            xt.astype(ml.float8_e4m3).transpose(1, 0, 2))
        wa = np.stack([
            (wK[h].astype(np.float64) @ wQ[h].astype(np.float64).T)
            .astype(np.float32).astype(bf).reshape(DC, P, D).transpose(1, 0, 2)
            for h in hs])
        wvo = np.stack([
            (wV[h].astype(np.float64)
             @ wO[h * D:(h + 1) * D, :].astype(np.float64))
            .astype(np.float32).astype(bf).reshape(DC, P, D).transpose(1, 0, 2)
            for h in hs])
        maps.append({
            "xt": xt,
            "xt8": xt8,
            "wa": np.ascontiguousarray(wa),
            "wvo": np.ascontiguousarray(wvo),
            "mask": mask,
        })
    return maps


def _run(inputs, wQ, wK, wV, wO, trace=False, tmpdir=None):
    import time

    from concourse.bass_utils import run_bass_kernel_spmd

    if "nc" not in _STATE:
        _STATE["nc"] = _build_nc()
    maps = _in_maps(inputs, wQ, wK, wV, wO)
    res = None
    for attempt in range(4):
        try:
            res = run_bass_kernel_spmd(_STATE["nc"], maps,
                                       list(range(N_CORES)),
                                       trace=trace, tmpdir=tmpdir)
            break
        except Exception:
            # Transient NRT device faults have been observed on the first
            # execution of a fresh executable; reset the backend and retry.
            if attempt == 3:
                raise
            try:
                import jax.extend.backend

                jax.extend.backend.clear_backends()
            except Exception:
                pass
            time.sleep(3.0)
    out = np.zeros((B, S, D), dtype=np.float32)
    for core in range(N_CORES):
        out[core // 4] += res.results[core]["out"]
    return out, res


def kernel(inputs, timestamp, wQ, wK, wV, wO, theta):
    inputs = np.asarray(inputs, dtype=np.float32)
    out, _ = _run(inputs, np.asarray(wQ, np.float32),
                  np.asarray(wK, np.float32), np.asarray(wV, np.float32),
                  np.asarray(wO, np.float32))
    return out


def kernel_profiled(inputs, timestamp, wQ, wK, wV, wO, theta, tmpdir=None):
    inputs = np.asarray(inputs, dtype=np.float32)
    out, res = _run(inputs, np.asarray(wQ, np.float32),
                    np.asarray(wK, np.float32), np.asarray(wV, np.float32),
                    np.asarray(wO, np.float32), trace=True, tmpdir=tmpdir)
    return out, res
